# revision 13
# baseline (speedup 1.0000x reference)
"""Trainium2 Bass kernel for nn_BoxCrossCategoryLoss (8-core data-parallel).

Math per row (36 terms): relu(pAB[i][:,f1] + pBC[j][:,f2] - c) where c is
pAC[k][:,1] (14 terms) or log1mexp(pAC[k][:,0]) (22 terms).  The int
*_rel_id inputs are unused by the reference and never uploaded.

Strategy (vs the previous kernel): loss = sum of relu over 7 "c-groups",
each computed as one broadcast tensor_tensor subtract over a permuted
S-slab followed by one fused relu+accumulate.  Work is LP-balanced over
three engines:
  ACT : 12 Exp + 13 Ln (the irreducible transcendentals) + relu of the
        two biggest groups (17 col-passes) with accum_out.
  DVE : all p-value/S adds (tensor_tensor bf16, 2x mode), subs for 6
        groups, relu+acc (tensor_scalar 4x) for 5 groups.
  Pool (gpsimd): the L2-group subtract (12 col-passes).
All Ln ops use bias = 1+2^-12 so log arguments stay > 0 (no -inf/NaN can
reach an accumulator); bf16(exp)=1.0 rows yield relu()=0 exactly as the
true term is ~0 there.  Per-partition partial sums land in fp32 stats,
DMA'd out and reduced on host in float64.
"""

import os
import sys

import numpy as np

for _p in ("/opt/trn_rl_repo", "/root/.axon_site/_ro/trn_rl_repo"):
    if os.path.isdir(_p) and _p not in sys.path:
        sys.path.insert(0, _p)

import ml_dtypes  # noqa: E402
import concourse.bacc as bacc  # noqa: E402
from concourse import mybir, tile  # noqa: E402
from concourse.bass_utils import run_bass_kernel_spmd  # noqa: E402

BF16 = ml_dtypes.bfloat16
F32 = mybir.dt.float32
BF = mybir.dt.bfloat16
Alu = mybir.AluOpType
Act = mybir.ActivationFunctionType

N_CORES = 8
P = 128
LNBIAS = 1.000244140625  # 1 + 2^-12: keeps every Ln argument positive

# Input column order inside the packed per-chunk slab.
# 0:AB0 1:AB1 2:BA0 3:BA1 4:BC0 5:BC1 6:CB0 7:CB1 8:AC1 9:CA1 10:AC0 11:CA0
COLS = ["AB0", "AB1", "BA0", "BA1", "BC0", "BC1", "CB0", "CB1",
        "AC1", "CA1", "AC0", "CA0"]
PAD_VAL = {"AB": -20.0, "BA": -20.0, "BC": -20.0, "CB": -20.0,
           "AC": -1e-3, "CA": -1e-3}

# A/B slab layout: [X00 X01 X10 X11 X20 X21 X31] (index 2k+c, X31 at 6)
# S-slab permuted order (positions chosen so every c-group is contiguous):
#   pos:  0  1  2  3  4 |  5  6  7  8  9 | 10 11 | 12 13
#   S  : S0 S1 S4 S8 S9 | S2 S3 S5 S10 S11 | S7 S13 | S6 S12
# S_i = A[a_i] + B[b_i]  (slab indices into A/B slabs)
S_AT_POS = [  # (A-slab idx, B-slab idx) per S-slab position
    (0, 1), (0, 5), (4, 1), (1, 0), (1, 4),      # S0 S1 S4 S8 S9
    (2, 3), (2, 5), (4, 3), (3, 2), (3, 4),      # S2 S3 S5 S10 S11
    (4, 6), (6, 4),                              # S7 S13
    (4, 5), (5, 4),                              # S6 S12
]
# groups: (name, slab_start_pos, n_terms, c_source)
# c_source: ('C', k) -> C-slab col k;  ('L01', k) -> LP01 col k; ('L2',)
# Engine split (LP-balanced): DVE sub+relu C01/C11/L0; DVE subs C31+C21
# with ACT relu; Pool subs L2 (ACT relu) and L1 (DVE relu).
G_C01 = ("C01", 0, 5, ("C", 0))
G_L1 = ("L1", 0, 5, ("L01", 1))
G_C11 = ("C11", 5, 5, ("C", 1))
G_L0 = ("L0", 5, 5, ("L01", 0))
G_C31 = ("C31", 10, 2, ("C", 3))
G_C21 = ("C21", 12, 2, ("C", 2))
G_L2 = ("L2", 0, 12, ("L2",))

DVE_FULL_GROUPS = [G_C01, G_C11, G_L0]   # sub+relu+acc on DVE, same round
DVE_SUB_ACT_RELU = [G_C31, G_C21]        # DVE sub -> dA56, ACT relu (deferred)
N_DVE_RELU = 4                           # C01 C11 L0 + deferred L1
N_ACT_RELU = 2                           # dA7 (L2), dA56 (C31+C21)


def build_module(F: int, rounds: int):
    nf = F * rounds
    nc = bacc.Bacc("TRN2", target_bir_lowering=False, debug=False,
                   enable_asserts=False, num_devices=N_CORES)
    # register the Ln bias constant (only 0.0/1.0 are pre-registered)
    _ct = nc.alloc_sbuf_tensor(f"const-f32-{LNBIAS}", [P, 1], F32)
    nc.gpsimd.memset(_ct.ap(), LNBIAS)
    nc.const_aps.aps[(F32, LNBIAS)] = _ct.ap()
    nc.all_engine_barrier()

    vin = nc.dram_tensor("vin", [P, 12 * nf], BF, kind="ExternalInput").ap()
    n_dve = N_DVE_RELU
    n_act = N_ACT_RELU
    out_d = nc.dram_tensor("stats_d", [P, n_dve * rounds], F32,
                           kind="ExternalOutput").ap()
    out_a = nc.dram_tensor("stats_a", [P, n_act * rounds], F32,
                           kind="ExternalOutput").ap()

    from contextlib import ExitStack
    with tile.TileContext(nc) as tc, ExitStack() as ctx:
        p1 = ctx.enter_context(tc.tile_pool(name="p1", bufs=1))
        p2 = ctx.enter_context(tc.tile_pool(name="p2", bufs=2))
        stp = ctx.enter_context(tc.tile_pool(name="st", bufs=1))

        st_d = stp.tile([P, n_dve * rounds], F32, tag="std")
        st_a = stp.tile([P, n_act * rounds], F32, tag="sta")

        def bcast(c_ap, n):
            return c_ap[:, None, :].broadcast_to([P, n, F])

        def view3(t_ap, n):
            return t_ap.rearrange("p (a b) -> p a b", a=n)

        pend = None  # deferred ACT relus from the previous round

        for k in range(rounds):
            IN = p2.tile([P, 12 * F], BF, tag="in")
            nc.sync.dma_start(IN[:], vin[:, k * 12 * F:(k + 1) * 12 * F])

            # ---------------- ACT: transcendentals ----------------
            EL = p2.tile([P, 2 * F], BF, tag="el")     # exp(AC0), exp(CA0)
            nc.scalar.activation(EL[:], IN[:, 10 * F:12 * F], Act.Exp)
            E8 = p1.tile([P, 8 * F], BF, tag="e8")
            nc.scalar.activation(E8[:], IN[:, 0:8 * F], Act.Exp)
            L8 = p1.tile([P, 8 * F], BF, tag="l8")
            nc.scalar.activation(L8[:], E8[:], Act.Ln, bias=LNBIAS, scale=-1.0)
            EC = p1.tile([P, 2 * F], F32, tag="ec")    # exp(AC1), exp(CA1)
            nc.scalar.activation(EC[:], IN[:, 8 * F:10 * F], Act.Exp)
            # EC is fp32 and strictly < 1, so bias=1.0 is exact and safe here
            LC = p2.tile([P, 2 * F], BF, tag="lc")     # lAC1, lCA1
            nc.scalar.activation(LC[:], EC[:], Act.Ln, bias=1.0, scale=-1.0)

            # ---------------- DVE/Pool: products for L terms ----------------
            M = p1.tile([P, 2 * F], BF, tag="m")       # 1-exp
            nc.vector.tensor_scalar(M[:], EL[:], -1.0, 1.0, Alu.mult, Alu.add)
            Pp = p2.tile([P, 2 * F], BF, tag="pp")     # P0, P1
            nc.vector.tensor_tensor(Pp[:, 0:F], EL[:, 0:F], M[:, F:2 * F],
                                    Alu.mult)
            nc.vector.tensor_tensor(Pp[:, F:2 * F], M[:, 0:F], EL[:, F:2 * F],
                                    Alu.mult)
            P2t = p2.tile([P, F], BF, tag="p2")        # P2
            nc.vector.tensor_tensor(P2t[:], EL[:, 0:F], EL[:, F:2 * F],
                                    Alu.mult)

            # ---------------- ACT: L-values ----------------
            LP01 = p2.tile([P, 2 * F], BF, tag="lp01")  # L0, L1
            nc.scalar.activation(LP01[:], Pp[:], Act.Ln, bias=LNBIAS,
                                 scale=-1.0)
            LL2 = p2.tile([P, F], BF, tag="ll2")        # L2
            nc.scalar.activation(LL2[:], P2t[:], Act.Ln, bias=LNBIAS,
                                 scale=-1.0)

            # ---------------- ACT: deferred relus from round k-1 ----------
            if pend is not None:
                _emit_act_relus(nc, pend)

            # ---------------- DVE: p-value adds ----------------
            A = p1.tile([P, 7 * F], BF, tag="A")
            B = p1.tile([P, 7 * F], BF, tag="B")
            for X, ofs in ((A, 0), (B, 4 * F)):
                v = IN[:, ofs:ofs + 4 * F]      # [v1c0 v1c1 v2c0 v2c1]
                l = L8[:, ofs:ofs + 4 * F]
                nc.vector.tensor_tensor(X[:, 0:2 * F], v[:, 0:2 * F],
                                        l[:, 2 * F:4 * F], Alu.add)  # X0c
                nc.vector.tensor_tensor(X[:, 2 * F:4 * F], l[:, 0:2 * F],
                                        v[:, 2 * F:4 * F], Alu.add)  # X1c
                nc.vector.tensor_tensor(X[:, 4 * F:6 * F], v[:, 0:2 * F],
                                        v[:, 2 * F:4 * F], Alu.add)  # X2c
                nc.vector.tensor_tensor(X[:, 6 * F:7 * F], l[:, F:2 * F],
                                        l[:, 3 * F:4 * F], Alu.add)  # X31
            C = p1.tile([P, 4 * F], BF, tag="C")       # C01 C11 C21 C31
            vA1, vC1 = IN[:, 8 * F:9 * F], IN[:, 9 * F:10 * F]
            lA1, lC1 = LC[:, 0:F], LC[:, F:2 * F]
            nc.vector.tensor_tensor(C[:, 0:F], vA1, lC1, Alu.add)
            nc.vector.tensor_tensor(C[:, F:2 * F], lA1, vC1, Alu.add)
            nc.vector.tensor_tensor(C[:, 2 * F:3 * F], vA1, vC1, Alu.add)
            nc.vector.tensor_tensor(C[:, 3 * F:4 * F], lA1, lC1, Alu.add)

            # ---------------- DVE: S-slab ----------------
            S = p1.tile([P, 14 * F], BF, tag="S")
            for pos, (ai, bi) in enumerate(S_AT_POS):
                nc.vector.tensor_tensor(S[:, pos * F:(pos + 1) * F],
                                        A[:, ai * F:(ai + 1) * F],
                                        B[:, bi * F:(bi + 1) * F], Alu.add)

            def c_ap(src):
                if src[0] == "C":
                    return C[:, src[1] * F:(src[1] + 1) * F]
                if src[0] == "L01":
                    return LP01[:, src[1] * F:(src[1] + 1) * F]
                return LL2[:]

            def sub_group(eng, grp, dst):
                _, pos, nt, src = grp
                eng.tensor_tensor(view3(dst, nt),
                                  view3(S[:, pos * F:(pos + nt) * F], nt),
                                  bcast(c_ap(src), nt), Alu.subtract)

            # ---------------- DVE: deferred L1 relu from round k-1 --------
            if pend is not None:
                _, kp, _, _, dP2p = pend
                nc.vector.tensor_scalar(
                    dP2p[:], dP2p[:], 0.0, None, Alu.max, Alu.add,
                    accum_out=st_d[:, kp * n_dve + 3:kp * n_dve + 4])
                pend = None
            # (emitted after S-adds so Pool has a full round of slack)

            # ---------------- DVE groups: sub + relu + acc ----------------
            dD = p1.tile([P, 5 * F], BF, tag="dD")
            for gi, grp in enumerate(DVE_FULL_GROUPS):
                nt = grp[2]
                d = dD[:, 0:nt * F]
                sub_group(nc.vector, grp, d)
                nc.vector.tensor_scalar(d, d, 0.0, None, Alu.max, Alu.add,
                                        accum_out=st_d[:, k * n_dve + gi:
                                                       k * n_dve + gi + 1])

            # ---------------- cross-engine groups ----------------
            dA56 = p2.tile([P, 4 * F], BF, tag="dA56")  # C31|C21, ACT relu
            sub_group(nc.vector, G_C31, dA56[:, 0:2 * F])
            sub_group(nc.vector, G_C21, dA56[:, 2 * F:4 * F])
            dA7 = p2.tile([P, 12 * F], BF, tag="dA7")   # L2 on Pool, ACT relu
            sub_group(nc.gpsimd, G_L2, dA7[:])
            dP2 = p2.tile([P, 5 * F], BF, tag="dP2")    # L1 on Pool, DVE relu
            sub_group(nc.gpsimd, G_L1, dP2[:])
            pend = (st_a, k, dA7, dA56, dP2)

        # tail: relus of the last round
        _emit_act_relus(nc, pend)
        _, kp, _, _, dP2p = pend
        nc.vector.tensor_scalar(dP2p[:], dP2p[:], 0.0, None, Alu.max, Alu.add,
                                accum_out=st_d[:, kp * n_dve + 3:
                                               kp * n_dve + 4])

        nc.sync.dma_start(out_d, st_d[:])
        nc.sync.dma_start(out_a, st_a[:])

    nc.compile()
    return nc


def _emit_act_relus(nc, pend):
    st_a, k, dA7, dA56, _ = pend
    nc.scalar.activation(dA7[:], dA7[:], Act.Relu,
                         accum_out=st_a[:, 2 * k:2 * k + 1])
    nc.scalar.activation(dA56[:], dA56[:], Act.Relu,
                         accum_out=st_a[:, 2 * k + 1:2 * k + 2])


_CACHE = {}


def _get_module(F, rounds):
    key = (F, rounds)
    if key not in _CACHE:
        _CACHE[key] = build_module(F, rounds)
    return _CACHE[key]


LAST_RESULTS = None  # BassKernelResults of the most recent run (for profiling)


def _plan(n_rows):
    per_core = -(-n_rows // (N_CORES * P))  # free elems per partition
    for F, rounds in ((872, 9), (784, 10), (712, 11), (656, 12), (560, 14)):
        if F * rounds >= per_core:
            best = (F, rounds)
    # pick the smallest nf that covers; iterate ascending instead
    cand = [(F, r) for (F, r) in ((872, 9), (784, 10), (712, 11), (656, 12),
                                  (560, 14)) if F * r >= per_core]
    cand.sort(key=lambda fr: fr[0] * fr[1])
    if cand:
        return cand[0]
    rounds = -(-per_core // 872)
    return 872, rounds


def kernel(**inputs) -> np.ndarray:
    global LAST_RESULTS
    vols = {X: np.asarray(inputs["vol_" + X]) for X in
            ["AB", "BA", "BC", "CB", "AC", "CA"]}
    n_rows = vols["AB"].shape[0]
    F, rounds = _plan(n_rows)
    nf = F * rounds
    total_rows = N_CORES * P * nf

    # full column stack in the packed order, padded
    cols = np.empty((12, total_rows), dtype=BF16)
    for j, name in enumerate(COLS):
        X, c = name[:2], int(name[2])
        col = np.full(total_rows, PAD_VAL[X], dtype=np.float32)
        col[:n_rows] = vols[X][:, c].astype(np.float32, copy=False)
        cols[j] = col.astype(BF16)
    # [12, cores, P, rounds, F] -> per core [P, rounds, 12, F]
    cview = cols.reshape(12, N_CORES, P, rounds, F)
    in_maps = []
    for core in range(N_CORES):
        pk = np.ascontiguousarray(
            cview[:, core].transpose(1, 2, 0, 3)).reshape(P, 12 * nf)
        in_maps.append({"vin": pk})

    nc = _get_module(F, rounds)
    trace = bool(os.environ.get("BASS_TRACE"))
    if trace:
        try:
            from antenv import axon_hooks  # noqa: F401
        except ImportError:
            trace = False
    if not trace:
        os.environ["BASS_NEVER_TRACE"] = "1"
    res = run_bass_kernel_spmd(nc, in_maps, core_ids=list(range(N_CORES)),
                               trace=trace)
    LAST_RESULTS = res
    total = np.float64(0.0)
    for om in res.results:
        total += om["stats_d"].astype(np.float64).sum()
        total += om["stats_a"].astype(np.float64).sum()
    return np.asarray(total, dtype=np.float32)


if __name__ == "__main__":
    rng = np.random.default_rng(0)
    n = 100_000
    ins = {}
    for X in ["AB", "BA", "BC", "CB", "AC", "CA"]:
        u = rng.uniform(1e-6, 1 - 1e-6, size=(n, 2)).astype(np.float32)
        ins["vol_" + X] = np.log(u)
    for nm in ("xy_rel_id", "yz_rel_id", "xz_rel_id"):
        ins[nm] = rng.integers(0, 2, size=(n, 2)).astype(np.int32)
    print("kernel:", kernel(**ins))

    # reference check on host
    def log1mexp(x):
        return np.log1p(-np.exp(x))
    DM = {0: 0, 1: 0, 2: 0, 3: 0, 4: 1, 5: 1, 6: 1, 7: 1}
    LR = [(0, 4, 4), (0, 6, 4), (1, 5, 5), (1, 6, 5), (2, 4, 4), (2, 5, 5),
          (2, 6, 6), (2, 7, 7), (4, 0, 4), (4, 2, 4), (5, 1, 5), (5, 2, 5),
          (6, 2, 6), (7, 2, 7)]
    NLR = [(0, 4, 1), (0, 4, 2), (0, 6, 1), (0, 6, 2), (1, 5, 0), (1, 5, 2),
           (1, 6, 0), (1, 6, 2), (2, 4, 1), (2, 4, 2), (2, 5, 0), (2, 5, 2),
           (4, 0, 1), (4, 0, 2), (4, 2, 1), (4, 2, 2), (5, 1, 0), (5, 1, 2),
           (5, 2, 0), (5, 2, 2), (2, 7, 2), (7, 2, 2)]

    def probs(v1, v2):
        l1, l2 = log1mexp(v1), log1mexp(v2)
        return [v1 + l2, l1 + v2, v1 + v2, l1 + l2]
    pAB = probs(ins["vol_AB"], ins["vol_BA"])
    pBC = probs(ins["vol_BC"], ins["vol_CB"])
    pAC = probs(ins["vol_AC"], ins["vol_CA"])
    loss = 0.0
    for xy, yz, xz in LR:
        t = (pAB[xy % 4][:, DM[xy]] + pBC[yz % 4][:, DM[yz]]
             - pAC[xz % 4][:, DM[xz]])
        loss += np.maximum(0, t).sum(dtype=np.float64)
    for xy, yz, xz in NLR:
        t = (pAB[xy % 4][:, DM[xy]] + pBC[yz % 4][:, DM[yz]]
             - log1mexp(pAC[xz % 4][:, DM[xz]]))
        loss += np.maximum(0, t).sum(dtype=np.float64)
    print("expected:", loss)
